# revision 23
# baseline (speedup 1.0000x reference)
"""Multi-head attention block for Trainium2, 8-core data-parallel SPMD.

Computes, per batch element b (one NeuronCore each):
    qkv = x @ w_qkv ; q,k,v split into 16 heads of dim 64
    attn = softmax(q @ k^T / sqrt(64)) ; out = (attn @ v) @ w_out + b_out

Design notes (v3, bf16):
  - All PE operands are bf16 (inputs are cast on the host): halves DMA
    and DVE traffic and gives LDWEIGHTS headroom to hide under matmuls.
    PSUM accumulation stays fp32.
  - The kernel is paced by the PE stream, which must stay *gapless*: the
    HAM clock gate only holds K=8/8 (2.4 GHz) while the PE never idles,
    and a single >3us bubble costs a 2x clock penalty for a long
    stretch.  Every off-PE chain (softmax normalize, projections'
    PSUM->SBUF casts) is arranged so the PE never waits on it.
  - Attention per head in transposed layout: s^T = kT^T @ qT, exp on ACT
    (one [128,1024] activation per (head, k-chunk) to amortize the
    ~350-cycle ACT fixed cost), o^T += vaug^T @ p^T with a ones column
    per head producing the softmax row-sums in the same matmuls.
  - AV matmuls trail the exp stream by LAG chunks (a global queue across
    head boundaries) so ACT latency never stalls the PE.
  - Softmax normalize is fully off the PE: the accumulator is staged to
    SBUF (freeing its PSUM bank for the next head), reciprocal via the
    fast approx custom-DVE op, partition-broadcast on GpSimd, final
    multiply on DVE.
  - PSUM budget (8 banks): sT [P,1024]x2 on the "mm" tag (4) + proj
    [P,512] (1) + acc [65,512]x3 rotation (3).  The "mm" tag is reused
    by transposes / v-proj / out-proj in the other phases.
  - q/k projections of pair t+1 are emitted interleaved into pair t's
    attention slots (one 512-wide accumulation step per slot), keeping
    the PE stream dense through the whole attention phase.
"""

import sys
from collections import deque

if "/opt/trn_rl_repo" not in sys.path:
    sys.path.insert(0, "/opt/trn_rl_repo")

import numpy as np

B = 8
N = 1024  # sequence length
C = 1024  # model dim
H = 16  # heads
D = 64  # head dim
P = 128  # partitions
NT = N // P  # seq chunks
CT = C // P  # channel chunks
HP = H // 2  # head pairs
SCALE = D ** -0.5
HF = C // 512  # 512-wide halves per 1024 row
LAG = 4  # AV matmuls trail the S/exp stream by this many k-chunks

_CACHE = {}


def _build_program():
    from concourse import bacc, mybir
    import concourse.tile as tile
    from concourse.masks import make_identity

    f32 = mybir.dt.float32
    bf16 = mybir.dt.bfloat16
    Exp = mybir.ActivationFunctionType.Exp

    nc = bacc.Bacc("TRN2", target_bir_lowering=False, debug=False)
    x_d = nc.declare_dram_parameter("x", [N, C], bf16, isOutput=False)
    wqkv_d = nc.declare_dram_parameter("w_qkv", [C, 3 * C], bf16, isOutput=False)
    wout_d = nc.declare_dram_parameter("w_out", [C, C], bf16, isOutput=False)
    bout_d = nc.declare_dram_parameter("b_out", [1, C], bf16, isOutput=False)
    out_d = nc.declare_dram_parameter("out", [N, C], f32, isOutput=True)

    with tile.TileContext(nc) as tc:
        with (
            tc.tile_pool(name="consts", bufs=1) as consts,
            tc.tile_pool(name="xTo", bufs=CT) as xT_pool,
            tc.tile_pool(name="vaug", bufs=NT) as vaug_pool,
            tc.tile_pool(name="psum", bufs=1, space="PSUM") as psum,
            tc.tile_pool(name="oTp", bufs=CT) as oT_pool,
            tc.tile_pool(name="io", bufs=3) as io_pool,
            tc.tile_pool(name="xin", bufs=NT) as xin_pool,
            tc.tile_pool(name="wv", bufs=CT) as wv_pool,
            tc.tile_pool(name="wo", bufs=CT) as wo_pool,
            tc.tile_pool(name="wqk", bufs=4) as wqk_pool,
            tc.tile_pool(name="qkT", bufs=4) as qkT_pool,
            tc.tile_pool(name="pT", bufs=LAG + 2) as pT_pool,
            tc.tile_pool(name="oTu", bufs=2) as oTu_pool,
            tc.tile_pool(name="rsum", bufs=2) as rs_pool,
            tc.tile_pool(name="recip", bufs=2) as rc_pool,
            tc.tile_pool(name="bcs", bufs=2) as bcs_pool,
        ):
            identity_f32 = consts.tile(
                [P, P], f32, name="identity_f32", tag="identity_f32"
            )
            make_identity(nc, identity_f32)
            identity = consts.tile([P, P], bf16, name="identity", tag="identity")
            nc.vector.tensor_copy(identity[:, :], identity_f32[:, :])
            ones_f32 = consts.tile([P, P], f32, name="ones_f32", tag="ones_f32")
            nc.vector.memset(ones_f32, 1.0)
            b_row = consts.tile([1, C], bf16, name="b_row", tag="b_row")
            nc.sync.dma_start(out=b_row[0:1, :], in_=bout_d[0:1, :])
            # bias broadcast to all partitions once; phase 3 then adds it on
            # DVE in the PSUM->SBUF move instead of 16 PE matmuls
            b_f32 = consts.tile([1, C], f32, name="b_f32", tag="b_f32")
            nc.vector.tensor_copy(b_f32[0:1, :], b_row[0:1, :])
            bias_bc = consts.tile([P, C], f32, name="bias_bc", tag="bias_bc")
            nc.gpsimd.partition_broadcast(bias_bc[:, :], b_f32[0:1, :], channels=P)

            xT = [
                xT_pool.tile([P, N], bf16, name=f"xT{i}", tag="xTo") for i in range(CT)
            ]
            vaug = [
                vaug_pool.tile([P, H * (D + 1)], bf16, name=f"vaug{i}", tag="vaug")
                for i in range(NT)
            ]
            oT = [
                oT_pool.tile([P, N], bf16, name=f"oT{i}", tag="oTp") for i in range(CT)
            ]

            def mm_tile(name, dtype=f32):
                return psum.tile([P, N], dtype, name=name, tag="mm", bufs=2)

            # ---------------- phase 0: transpose x into xT ----------------
            xins = []
            for si in range(NT):
                xin = xin_pool.tile([P, N], bf16, name=f"xin{si}", tag="xin")
                nc.sync.dma_start(out=xin[:, :], in_=x_d[si * P : (si + 1) * P, :])
                xins.append(xin)
            for si in range(NT):
                xin = xins[si]
                tr_ps = mm_tile(f"tr{si}", bf16)
                for ci in range(CT):
                    nc.tensor.transpose(
                        tr_ps[:, ci * P : (ci + 1) * P],
                        xin[:, ci * P : (ci + 1) * P],
                        identity,
                    )
                for ci in range(CT):
                    nc.vector.tensor_copy(
                        xT[ci][:, si * P : (si + 1) * P],
                        tr_ps[:, ci * P : (ci + 1) * P],
                    )

            # ---- interleavable q/k projection generator (one pair) ----
            def pair_proj_gen(t, qTt, kTt):
                """Yields once per 512-wide accumulation step (32 total);
                prefetches all weight chunks at creation; finishes with the
                PSUM->SBUF casts emitted between sub-phases."""
                wtiles = {}
                for which, colbase in (("q", t * P), ("k", C + t * P)):
                    # all 8 contraction chunks of this weight column block in
                    # one strided DMA (row ci*128+p, col c -> [p, ci, c])
                    w = wqk_pool.tile([P, CT * P], bf16, name=f"w{which}{t}", tag="wqk")
                    nc.sync.dma_start(
                        out=w.rearrange("p (ci c) -> p ci c", c=P),
                        in_=wqkv_d[:, colbase : colbase + P].rearrange(
                            "(ci p) c -> p ci c", p=P
                        ),
                    )
                    wtiles[which] = w
                for which, dst in (("q", qTt), ("k", kTt)):
                    for sh in range(HF):
                        sl = slice(sh * 512, sh * 512 + 512)
                        ps = psum.tile(
                            [P, 512], f32, name=f"pj{t}{which}{sh}", tag="proj", bufs=2
                        )
                        for ci in range(CT):
                            nc.tensor.matmul(
                                ps[:, :],
                                wtiles[which][:, ci * P : (ci + 1) * P],
                                xT[ci][:, sl],
                                start=(ci == 0),
                                stop=(ci == CT - 1),
                            )
                            yield
                        nc.vector.tensor_copy(dst[:, sl], ps[:, :])

            def new_pair(t):
                qTt = qkT_pool.tile([P, N], bf16, name=f"qT{t}", tag="qkT")
                kTt = qkT_pool.tile([P, N], bf16, name=f"kT{t}", tag="qkT")
                return qTt, kTt, pair_proj_gen(t, qTt, kTt)

            # -------- phase 1: v projection (+ pair-0 q/k interleaved) --------
            wv = []
            for ci in range(CT):
                w = wv_pool.tile([P, N], bf16, name=f"wv{ci}", tag="wv")
                nc.sync.dma_start(
                    out=w[:, :], in_=wqkv_d[ci * P : (ci + 1) * P, 2 * C : 3 * C]
                )
                wv.append(w)

            qT_cur, kT_cur, gen0 = new_pair(0)
            for sc in range(NT):
                v_ps = mm_tile(f"vps{sc}")
                for ci in range(CT):
                    st = dict(start=(ci == 0), stop=(ci == CT - 1))
                    for hf in range(HF):
                        sl = slice(hf * 512, hf * 512 + 512)
                        nc.tensor.matmul(
                            v_ps[:, sl],
                            xT[ci][:, sc * P : (sc + 1) * P],
                            wv[ci][:, sl],
                            **st,
                        )
                    next(gen0, None)
                va3 = vaug[sc].rearrange("p (h u) -> p h u", u=D + 1)
                nc.vector.tensor_copy(
                    va3[:, :, D : D + 1],
                    ones_f32[:, 0:H].rearrange("p (h u) -> p h u", u=1),
                )
                nc.vector.tensor_copy(
                    va3[:, :, 0:D],
                    v_ps.rearrange("p (h u) -> p h u", u=D),
                )
            for _ in gen0:  # drain any remaining projection steps
                pass

            # ---------------- phase 2: attention ----------------
            wos = []

            def prefetch_wo(ci):
                wo = wo_pool.tile([P, N], bf16, name=f"wo{ci}", tag="wo")
                nc.sync.dma_start(out=wo[:, :], in_=wout_d[ci * P : (ci + 1) * P, :])
                wos.append(wo)

            def normalize(h, t, row0, accs):
                # stage the accumulator out of PSUM (frees the acc banks),
                # then 1/rowsum (approx) -> partition-broadcast -> multiply.
                oTu = oTu_pool.tile([D + 1, N], f32, name=f"oTu{h}", tag="oTu")
                rs = rs_pool.tile([1, N], f32, name=f"rs{h}", tag="rsum")
                for hf in range(HF):
                    sl = slice(hf * 512, hf * 512 + 512)
                    nc.vector.tensor_copy(oTu[0 : D + 1, sl], accs[hf][0 : D + 1, :])
                    nc.vector.tensor_copy(rs[0:1, sl], accs[hf][D : D + 1, :])
                rc = rc_pool.tile([1, N], f32, name=f"rc{h}", tag="recip")
                nc.vector.reciprocal_approx_fast(rc[0:1, :], rs[0:1, :])
                bcs = bcs_pool.tile([D, N], f32, name=f"bcs{h}", tag="bcs")
                nc.gpsimd.partition_broadcast(bcs[0:D, :], rc[0:1, :], channels=D)
                nc.vector.tensor_mul(
                    oT[t][row0 : row0 + D, :], oTu[0:D, :], bcs[0:D, :]
                )

            av_queue = deque()  # (emit_fn, head_done_fn | None)

            def drain_av(n):
                for _ in range(n):
                    if not av_queue:
                        return
                    emit, done = av_queue.popleft()
                    emit()
                    if done is not None:
                        done()

            for t in range(HP):
                prefetch_wo(t)
                if t + 1 < HP:
                    qT_nxt, kT_nxt, gen = new_pair(t + 1)
                else:
                    qT_nxt = kT_nxt = gen = None
                for j in range(2):
                    h = 2 * t + j
                    row0 = D * j
                    accs = [
                        psum.tile([P, 512], f32, name=f"acc{h}_{hf}", tag="acc", bufs=2)
                        for hf in range(HF)
                    ]

                    def make_av(kc, pt, accs=accs, h=h):
                        def emit():
                            for hf in range(HF):
                                sl = slice(hf * 512, hf * 512 + 512)
                                nc.tensor.matmul(
                                    accs[hf][0 : D + 1, :],
                                    vaug[kc][:, h * (D + 1) : (h + 1) * (D + 1)],
                                    pt[:, sl],
                                    start=(kc == 0),
                                    stop=(kc == NT - 1),
                                )

                        return emit

                    done_fn = (
                        lambda h=h, t=t, row0=row0, accs=accs: normalize(
                            h, t, row0, accs
                        )
                    )
                    for kc in range(NT):
                        # trailing AV + projection filler go first so the S
                        # matmuls' exp-rotation dependency gets a full slot
                        # of extra slack before the PE reaches them
                        if len(av_queue) >= LAG:
                            drain_av(len(av_queue) - LAG + 1)
                        if gen is not None:
                            # 32 projection steps per pair over 16 slots
                            next(gen, None)
                            next(gen, None)
                        sT = mm_tile(f"s{h}_{kc}")
                        pt = pT_pool.tile([P, N], bf16, name=f"pt{h}_{kc}", tag="pT")
                        for hf in range(HF):
                            sl = slice(hf * 512, hf * 512 + 512)
                            nc.tensor.matmul(
                                sT[:, sl],
                                kT_cur[row0 : row0 + D, kc * P : (kc + 1) * P],
                                qT_cur[row0 : row0 + D, sl],
                                start=True,
                                stop=True,
                            )
                            nc.scalar.activation(
                                out=pt[:, sl], in_=sT[:, sl], func=Exp, scale=SCALE
                            )
                        av_queue.append(
                            (make_av(kc, pt), done_fn if kc == NT - 1 else None)
                        )
                if gen is not None:
                    for _ in gen:
                        pass
                qT_cur, kT_cur = qT_nxt, kT_nxt
            drain_av(len(av_queue))  # last head's AV tail + its normalize

            # ---------------- phase 3: out = o @ w_out + b ----------------
            for sc in range(NT):
                o_ps = mm_tile(f"ops{sc}")
                for ci in range(CT):
                    for hf in range(HF):
                        sl = slice(hf * 512, hf * 512 + 512)
                        nc.tensor.matmul(
                            o_ps[:, sl],
                            oT[ci][:, sc * P : (sc + 1) * P],
                            wos[ci][:, sl],
                            start=(ci == 0),
                            stop=(ci == CT - 1),
                        )
                ot = io_pool.tile([P, C], f32, name=f"ot{sc}", tag="ot")
                nc.vector.tensor_add(ot[:, :], o_ps[:, :], bias_bc[:, :])
                nc.sync.dma_start(out=out_d[sc * P : (sc + 1) * P, :], in_=ot[:, :])

    nc.compile()
    return nc


def _get_program():
    if "nc" not in _CACHE:
        _CACHE["nc"] = _build_program()
    return _CACHE["nc"]


def _bf16(a):
    import ml_dtypes

    return np.ascontiguousarray(np.asarray(a, dtype=np.float32)).astype(
        ml_dtypes.bfloat16
    )


def _in_maps(inputs):
    x = _bf16(inputs["x"])
    w_qkv = _bf16(inputs["w_qkv"])
    w_out = _bf16(inputs["w_out"])
    b_row = _bf16(np.asarray(inputs["b_out"]).reshape(1, C))
    return [
        {"x": x[i], "w_qkv": w_qkv, "w_out": w_out, "b_out": b_row} for i in range(B)
    ]


def kernel(x, w_qkv, w_out, b_out):
    from concourse.bass_utils import run_bass_kernel_spmd

    nc = _get_program()
    in_maps = _in_maps({"x": x, "w_qkv": w_qkv, "w_out": w_out, "b_out": b_out})
    res = run_bass_kernel_spmd(nc, in_maps, core_ids=list(range(B))).results
    return np.stack([res[i]["out"] for i in range(B)], axis=0)


# revision 24
# speedup vs baseline: 1.0714x; 1.0714x over previous
"""Multi-head attention block for Trainium2, 8-core data-parallel SPMD.

Computes, per batch element b (one NeuronCore each):
    qkv = x @ w_qkv ; q,k,v split into 16 heads of dim 64
    attn = softmax(q @ k^T / sqrt(64)) ; out = (attn @ v) @ w_out + b_out

Design notes (v3, bf16):
  - All PE operands are bf16 (inputs are cast on the host): halves DMA
    and DVE traffic and gives LDWEIGHTS headroom to hide under matmuls.
    PSUM accumulation stays fp32.
  - The kernel is paced by the PE stream, which must stay *gapless*: the
    HAM clock gate only holds K=8/8 (2.4 GHz) while the PE never idles,
    and a single >3us bubble costs a 2x clock penalty for a long
    stretch.  Every off-PE chain (softmax normalize, projections'
    PSUM->SBUF casts) is arranged so the PE never waits on it.
  - Attention per head in transposed layout: s^T = kT^T @ qT, exp on ACT
    (one [128,1024] activation per (head, k-chunk) to amortize the
    ~350-cycle ACT fixed cost), o^T += vaug^T @ p^T with a ones column
    per head producing the softmax row-sums in the same matmuls.
  - AV matmuls trail the exp stream by LAG chunks (a global queue across
    head boundaries) so ACT latency never stalls the PE.
  - Softmax normalize is fully off the PE: the accumulator is staged to
    SBUF (freeing its PSUM bank for the next head), reciprocal via the
    fast approx custom-DVE op, partition-broadcast on GpSimd, final
    multiply on DVE.
  - PSUM budget (8 banks): sT [P,1024]x2 on the "mm" tag (4) + proj
    [P,512] (1) + acc [65,512]x3 rotation (3).  The "mm" tag is reused
    by transposes / v-proj / out-proj in the other phases.
  - q/k projections of pair t+1 are emitted interleaved into pair t's
    attention slots (one 512-wide accumulation step per slot), keeping
    the PE stream dense through the whole attention phase.
"""

import sys
from collections import deque

if "/opt/trn_rl_repo" not in sys.path:
    sys.path.insert(0, "/opt/trn_rl_repo")

import numpy as np

B = 8
N = 1024  # sequence length
C = 1024  # model dim
H = 16  # heads
D = 64  # head dim
P = 128  # partitions
NT = N // P  # seq chunks
CT = C // P  # channel chunks
HP = H // 2  # head pairs
SCALE = D ** -0.5
HF = C // 512  # 512-wide halves per 1024 row
LAG = 4  # AV matmuls trail the S/exp stream by this many k-chunks

_CACHE = {}


def _build_program():
    from concourse import bacc, mybir
    import concourse.tile as tile
    from concourse.masks import make_identity

    f32 = mybir.dt.float32
    bf16 = mybir.dt.bfloat16
    Exp = mybir.ActivationFunctionType.Exp

    nc = bacc.Bacc("TRN2", target_bir_lowering=False, debug=False)
    x_d = nc.declare_dram_parameter("x", [N, C], bf16, isOutput=False)
    wqkv_d = nc.declare_dram_parameter("w_qkv", [C, 3 * C], bf16, isOutput=False)
    wout_d = nc.declare_dram_parameter("w_out", [C, C], bf16, isOutput=False)
    bout_d = nc.declare_dram_parameter("b_out", [1, C], bf16, isOutput=False)
    out_d = nc.declare_dram_parameter("out", [N, C], f32, isOutput=True)

    with tile.TileContext(nc) as tc:
        with (
            tc.tile_pool(name="consts", bufs=1) as consts,
            tc.tile_pool(name="xTo", bufs=CT) as xT_pool,
            tc.tile_pool(name="vaug", bufs=NT) as vaug_pool,
            tc.tile_pool(name="psum", bufs=1, space="PSUM") as psum,
            tc.tile_pool(name="oTp", bufs=CT) as oT_pool,
            tc.tile_pool(name="io", bufs=3) as io_pool,
            tc.tile_pool(name="xin", bufs=NT) as xin_pool,
            tc.tile_pool(name="wv", bufs=CT) as wv_pool,
            tc.tile_pool(name="wo", bufs=CT) as wo_pool,
            tc.tile_pool(name="wqk", bufs=4) as wqk_pool,
            tc.tile_pool(name="qkT", bufs=4) as qkT_pool,
            tc.tile_pool(name="pT", bufs=LAG + 2) as pT_pool,
            tc.tile_pool(name="oTu", bufs=2) as oTu_pool,
            tc.tile_pool(name="rsum", bufs=2) as rs_pool,
            tc.tile_pool(name="recip", bufs=2) as rc_pool,
            tc.tile_pool(name="bcs", bufs=2) as bcs_pool,
        ):
            identity_f32 = consts.tile(
                [P, P], f32, name="identity_f32", tag="identity_f32"
            )
            make_identity(nc, identity_f32)
            identity = consts.tile([P, P], bf16, name="identity", tag="identity")
            nc.vector.tensor_copy(identity[:, :], identity_f32[:, :])
            ones_f32 = consts.tile([P, P], f32, name="ones_f32", tag="ones_f32")
            nc.vector.memset(ones_f32, 1.0)
            b_row = consts.tile([1, C], bf16, name="b_row", tag="b_row")
            nc.sync.dma_start(out=b_row[0:1, :], in_=bout_d[0:1, :])
            # bias broadcast to all partitions once; phase 3 then adds it on
            # DVE in the PSUM->SBUF move instead of 16 PE matmuls
            b_f32 = consts.tile([1, C], f32, name="b_f32", tag="b_f32")
            nc.vector.tensor_copy(b_f32[0:1, :], b_row[0:1, :])
            bias_bc = consts.tile([P, C], f32, name="bias_bc", tag="bias_bc")
            nc.gpsimd.partition_broadcast(bias_bc[:, :], b_f32[0:1, :], channels=P)

            xT = [
                xT_pool.tile([P, N], bf16, name=f"xT{i}", tag="xTo") for i in range(CT)
            ]
            vaug = [
                vaug_pool.tile([P, H * (D + 1)], bf16, name=f"vaug{i}", tag="vaug")
                for i in range(NT)
            ]
            oT = [
                oT_pool.tile([P, N], bf16, name=f"oT{i}", tag="oTp") for i in range(CT)
            ]

            def mm_tile(name, dtype=f32):
                return psum.tile([P, N], dtype, name=name, tag="mm", bufs=2)

            # ---------------- phase 0: transpose x into xT ----------------
            xins = []
            for si in range(NT):
                xin = xin_pool.tile([P, N], bf16, name=f"xin{si}", tag="xin")
                nc.sync.dma_start(out=xin[:, :], in_=x_d[si * P : (si + 1) * P, :])
                xins.append(xin)
            for si in range(NT):
                xin = xins[si]
                tr_ps = mm_tile(f"tr{si}", bf16)
                for ci in range(CT):
                    nc.tensor.transpose(
                        tr_ps[:, ci * P : (ci + 1) * P],
                        xin[:, ci * P : (ci + 1) * P],
                        identity,
                    )
                for ci in range(CT):
                    nc.vector.tensor_copy(
                        xT[ci][:, si * P : (si + 1) * P],
                        tr_ps[:, ci * P : (ci + 1) * P],
                    )

            # ---- interleavable q/k projection generator (one pair) ----
            def pair_proj_gen(t, qTt, kTt):
                """Yields once per 512-wide accumulation step (32 total);
                prefetches all weight chunks at creation; finishes with the
                PSUM->SBUF casts emitted between sub-phases."""
                wtiles = {}
                for which, colbase in (("q", t * P), ("k", C + t * P)):
                    # all 8 contraction chunks of this weight column block in
                    # one strided DMA (row ci*128+p, col c -> [p, ci, c])
                    w = wqk_pool.tile([P, CT * P], bf16, name=f"w{which}{t}", tag="wqk")
                    nc.sync.dma_start(
                        out=w.rearrange("p (ci c) -> p ci c", c=P),
                        in_=wqkv_d[:, colbase : colbase + P].rearrange(
                            "(ci p) c -> p ci c", p=P
                        ),
                    )
                    wtiles[which] = w
                for which, dst in (("q", qTt), ("k", kTt)):
                    for sh in range(HF):
                        sl = slice(sh * 512, sh * 512 + 512)
                        ps = psum.tile(
                            [P, 512], f32, name=f"pj{t}{which}{sh}", tag="proj", bufs=2
                        )
                        for ci in range(CT):
                            nc.tensor.matmul(
                                ps[:, :],
                                wtiles[which][:, ci * P : (ci + 1) * P],
                                xT[ci][:, sl],
                                start=(ci == 0),
                                stop=(ci == CT - 1),
                            )
                            yield
                        nc.vector.tensor_copy(dst[:, sl], ps[:, :])

            def new_pair(t):
                qTt = qkT_pool.tile([P, N], bf16, name=f"qT{t}", tag="qkT")
                kTt = qkT_pool.tile([P, N], bf16, name=f"kT{t}", tag="qkT")
                return qTt, kTt, pair_proj_gen(t, qTt, kTt)

            # -------- phase 1: v projection (+ pair-0 q/k interleaved) --------
            wv = []
            for ci in range(CT):
                w = wv_pool.tile([P, N], bf16, name=f"wv{ci}", tag="wv")
                nc.sync.dma_start(
                    out=w[:, :], in_=wqkv_d[ci * P : (ci + 1) * P, 2 * C : 3 * C]
                )
                wv.append(w)

            qT_cur, kT_cur, gen0 = new_pair(0)
            for sc in range(NT):
                v_ps = mm_tile(f"vps{sc}")
                for ci in range(CT):
                    st = dict(start=(ci == 0), stop=(ci == CT - 1))
                    for hf in range(HF):
                        sl = slice(hf * 512, hf * 512 + 512)
                        nc.tensor.matmul(
                            v_ps[:, sl],
                            xT[ci][:, sc * P : (sc + 1) * P],
                            wv[ci][:, sl],
                            **st,
                        )
                    next(gen0, None)
                va3 = vaug[sc].rearrange("p (h u) -> p h u", u=D + 1)
                nc.vector.tensor_copy(
                    va3[:, :, D : D + 1],
                    ones_f32[:, 0:H].rearrange("p (h u) -> p h u", u=1),
                )
                nc.vector.tensor_copy(
                    va3[:, :, 0:D],
                    v_ps.rearrange("p (h u) -> p h u", u=D),
                )
            for _ in gen0:  # drain any remaining projection steps
                pass

            # ---------------- phase 2: attention ----------------
            wos = []

            def prefetch_wo(ci):
                wo = wo_pool.tile([P, N], bf16, name=f"wo{ci}", tag="wo")
                nc.sync.dma_start(out=wo[:, :], in_=wout_d[ci * P : (ci + 1) * P, :])
                wos.append(wo)

            def normalize(h, t, row0, accs):
                # stage the accumulator out of PSUM (frees the acc banks),
                # then 1/rowsum (approx) -> partition-broadcast -> multiply.
                oTu = oTu_pool.tile([D + 1, N], f32, name=f"oTu{h}", tag="oTu")
                rs = rs_pool.tile([1, N], f32, name=f"rs{h}", tag="rsum")
                for hf in range(HF):
                    sl = slice(hf * 512, hf * 512 + 512)
                    nc.vector.tensor_copy(oTu[0 : D + 1, sl], accs[hf][0 : D + 1, :])
                    nc.vector.tensor_copy(rs[0:1, sl], accs[hf][D : D + 1, :])
                rc = rc_pool.tile([1, N], f32, name=f"rc{h}", tag="recip")
                nc.vector.reciprocal_approx_fast(rc[0:1, :], rs[0:1, :])
                bcs = bcs_pool.tile([D, N], f32, name=f"bcs{h}", tag="bcs")
                nc.gpsimd.partition_broadcast(bcs[0:D, :], rc[0:1, :], channels=D)
                nc.vector.tensor_mul(
                    oT[t][row0 : row0 + D, :], oTu[0:D, :], bcs[0:D, :]
                )

            av_queue = deque()  # (emit_fn, head_done_fn | None)

            def drain_av(n):
                for _ in range(n):
                    if not av_queue:
                        return
                    emit, done = av_queue.popleft()
                    emit()
                    if done is not None:
                        done()

            for t in range(HP):
                prefetch_wo(t)
                if t + 1 < HP:
                    qT_nxt, kT_nxt, gen = new_pair(t + 1)
                else:
                    qT_nxt = kT_nxt = gen = None
                for j in range(2):
                    h = 2 * t + j
                    row0 = D * j
                    accs = [
                        psum.tile([P, 512], f32, name=f"acc{h}_{hf}", tag="acc", bufs=2)
                        for hf in range(HF)
                    ]

                    def make_av(kc, pt, accs=accs, h=h):
                        def emit():
                            for hf in range(HF):
                                sl = slice(hf * 512, hf * 512 + 512)
                                nc.tensor.matmul(
                                    accs[hf][0 : D + 1, :],
                                    vaug[kc][:, h * (D + 1) : (h + 1) * (D + 1)],
                                    pt[:, sl],
                                    start=(kc == 0),
                                    stop=(kc == NT - 1),
                                )

                        return emit

                    done_fn = (
                        lambda h=h, t=t, row0=row0, accs=accs: normalize(
                            h, t, row0, accs
                        )
                    )
                    for kc in range(NT):
                        sT = mm_tile(f"s{h}_{kc}")
                        for hf in range(HF):
                            sl = slice(hf * 512, hf * 512 + 512)
                            nc.tensor.matmul(
                                sT[:, sl],
                                kT_cur[row0 : row0 + D, kc * P : (kc + 1) * P],
                                qT_cur[row0 : row0 + D, sl],
                                start=True,
                                stop=True,
                            )
                        pt = pT_pool.tile([P, N], bf16, name=f"pt{h}_{kc}", tag="pT")
                        for hf in range(HF):
                            sl = slice(hf * 512, hf * 512 + 512)
                            nc.scalar.activation(
                                out=pt[:, sl], in_=sT[:, sl], func=Exp, scale=SCALE
                            )
                        av_queue.append(
                            (make_av(kc, pt), done_fn if kc == NT - 1 else None)
                        )
                        if len(av_queue) > LAG:
                            drain_av(len(av_queue) - LAG)
                        if gen is not None:
                            # 32 projection steps per pair over 16 slots
                            next(gen, None)
                            next(gen, None)
                if gen is not None:
                    for _ in gen:
                        pass
                qT_cur, kT_cur = qT_nxt, kT_nxt
            drain_av(len(av_queue))  # last head's AV tail + its normalize

            # ---------------- phase 3: out = o @ w_out + b ----------------
            for sc in range(NT):
                o_ps = mm_tile(f"ops{sc}")
                for ci in range(CT):
                    for hf in range(HF):
                        sl = slice(hf * 512, hf * 512 + 512)
                        nc.tensor.matmul(
                            o_ps[:, sl],
                            oT[ci][:, sc * P : (sc + 1) * P],
                            wos[ci][:, sl],
                            start=(ci == 0),
                            stop=(ci == CT - 1),
                        )
                ot = io_pool.tile([P, C], f32, name=f"ot{sc}", tag="ot")
                nc.vector.tensor_add(ot[:, :], o_ps[:, :], bias_bc[:, :])
                nc.sync.dma_start(out=out_d[sc * P : (sc + 1) * P, :], in_=ot[:, :])

    nc.compile()
    return nc


def _get_program():
    if "nc" not in _CACHE:
        _CACHE["nc"] = _build_program()
    return _CACHE["nc"]


def _bf16(a):
    import ml_dtypes

    return np.ascontiguousarray(np.asarray(a, dtype=np.float32)).astype(
        ml_dtypes.bfloat16
    )


def _in_maps(inputs):
    x = _bf16(inputs["x"])
    w_qkv = _bf16(inputs["w_qkv"])
    w_out = _bf16(inputs["w_out"])
    b_row = _bf16(np.asarray(inputs["b_out"]).reshape(1, C))
    return [
        {"x": x[i], "w_qkv": w_qkv, "w_out": w_out, "b_out": b_row} for i in range(B)
    ]


def kernel(x, w_qkv, w_out, b_out):
    from concourse.bass_utils import run_bass_kernel_spmd

    nc = _get_program()
    in_maps = _in_maps({"x": x, "w_qkv": w_qkv, "w_out": w_out, "b_out": b_out})
    res = run_bass_kernel_spmd(nc, in_maps, core_ids=list(range(B))).results
    return np.stack([res[i]["out"] for i in range(B)], axis=0)


# revision 26
# speedup vs baseline: 1.0723x; 1.0008x over previous
"""Multi-head attention block for Trainium2, 8-core data-parallel SPMD.

Computes, per batch element b (one NeuronCore each):
    qkv = x @ w_qkv ; q,k,v split into 16 heads of dim 64
    attn = softmax(q @ k^T / sqrt(64)) ; out = (attn @ v) @ w_out + b_out

Design notes (v3, bf16):
  - All PE operands are bf16 (inputs are cast on the host): halves DMA
    and DVE traffic and gives LDWEIGHTS headroom to hide under matmuls.
    PSUM accumulation stays fp32.
  - The kernel is paced by the PE stream, which must stay *gapless*: the
    HAM clock gate only holds K=8/8 (2.4 GHz) while the PE never idles,
    and a single >3us bubble costs a 2x clock penalty for a long
    stretch.  Every off-PE chain (softmax normalize, projections'
    PSUM->SBUF casts) is arranged so the PE never waits on it.
  - Attention per head in transposed layout: s^T = kT^T @ qT, exp on ACT
    (one [128,1024] activation per (head, k-chunk) to amortize the
    ~350-cycle ACT fixed cost), o^T += vaug^T @ p^T with a ones column
    per head producing the softmax row-sums in the same matmuls.
  - AV matmuls trail the exp stream by LAG chunks (a global queue across
    head boundaries) so ACT latency never stalls the PE.
  - Softmax normalize is fully off the PE: the accumulator is staged to
    SBUF (freeing its PSUM bank for the next head), reciprocal via the
    fast approx custom-DVE op, partition-broadcast on GpSimd, final
    multiply on DVE.
  - PSUM budget (8 banks): sT [P,1024]x2 on the "mm" tag (4) + proj
    [P,512]x2 (2) + acc [65,512]x2 rotation (2).  The "mm" tag is
    reused by transposes / v-proj / out-proj in the other phases.
  - q/k projections of pair t+1 are emitted interleaved into pair t's
    attention slots (one 512-wide accumulation step per slot), keeping
    the PE stream dense through the whole attention phase.
"""

import sys
from collections import deque

if "/opt/trn_rl_repo" not in sys.path:
    sys.path.insert(0, "/opt/trn_rl_repo")

import numpy as np

B = 8
N = 1024  # sequence length
C = 1024  # model dim
H = 16  # heads
D = 64  # head dim
P = 128  # partitions
NT = N // P  # seq chunks
CT = C // P  # channel chunks
HP = H // 2  # head pairs
SCALE = D ** -0.5
HF = C // 512  # 512-wide halves per 1024 row
LAG = 4  # AV matmuls trail the S/exp stream by this many k-chunks

_CACHE = {}


def _build_program():
    from concourse import bacc, mybir
    import concourse.tile as tile
    from concourse.masks import make_identity

    f32 = mybir.dt.float32
    bf16 = mybir.dt.bfloat16
    Exp = mybir.ActivationFunctionType.Exp

    nc = bacc.Bacc("TRN2", target_bir_lowering=False, debug=False)
    x_d = nc.declare_dram_parameter("x", [N, C], bf16, isOutput=False)
    wqkv_d = nc.declare_dram_parameter("w_qkv", [C, 3 * C], bf16, isOutput=False)
    wout_d = nc.declare_dram_parameter("w_out", [C, C], bf16, isOutput=False)
    bout_d = nc.declare_dram_parameter("b_out", [1, C], bf16, isOutput=False)
    out_d = nc.declare_dram_parameter("out", [N, C], f32, isOutput=True)

    with tile.TileContext(nc) as tc:
        with (
            tc.tile_pool(name="consts", bufs=1) as consts,
            tc.tile_pool(name="xTo", bufs=CT) as xT_pool,
            tc.tile_pool(name="vaug", bufs=NT) as vaug_pool,
            tc.tile_pool(name="psum", bufs=1, space="PSUM") as psum,
            tc.tile_pool(name="oTp", bufs=CT) as oT_pool,
            tc.tile_pool(name="io", bufs=3) as io_pool,
            tc.tile_pool(name="xin", bufs=NT) as xin_pool,
            tc.tile_pool(name="wv", bufs=CT) as wv_pool,
            tc.tile_pool(name="wo", bufs=CT) as wo_pool,
            tc.tile_pool(name="wqk", bufs=4) as wqk_pool,
            tc.tile_pool(name="qkT", bufs=4) as qkT_pool,
            tc.tile_pool(name="pT", bufs=LAG + 2) as pT_pool,
            tc.tile_pool(name="oTu", bufs=2) as oTu_pool,
            tc.tile_pool(name="rsum", bufs=2) as rs_pool,
            tc.tile_pool(name="recip", bufs=2) as rc_pool,
            tc.tile_pool(name="bcs", bufs=2) as bcs_pool,
        ):
            identity_f32 = consts.tile(
                [P, P], f32, name="identity_f32", tag="identity_f32"
            )
            make_identity(nc, identity_f32)
            identity = consts.tile([P, P], bf16, name="identity", tag="identity")
            nc.vector.tensor_copy(identity[:, :], identity_f32[:, :])
            ones_f32 = consts.tile([P, P], f32, name="ones_f32", tag="ones_f32")
            nc.vector.memset(ones_f32, 1.0)
            b_row = consts.tile([1, C], bf16, name="b_row", tag="b_row")
            nc.sync.dma_start(out=b_row[0:1, :], in_=bout_d[0:1, :])
            # bias broadcast to all partitions once; phase 3 then adds it on
            # DVE in the PSUM->SBUF move instead of 16 PE matmuls
            b_f32 = consts.tile([1, C], f32, name="b_f32", tag="b_f32")
            nc.vector.tensor_copy(b_f32[0:1, :], b_row[0:1, :])
            bias_bc = consts.tile([P, C], f32, name="bias_bc", tag="bias_bc")
            nc.gpsimd.partition_broadcast(bias_bc[:, :], b_f32[0:1, :], channels=P)

            xT = [
                xT_pool.tile([P, N], bf16, name=f"xT{i}", tag="xTo") for i in range(CT)
            ]
            vaug = [
                vaug_pool.tile([P, H * (D + 1)], bf16, name=f"vaug{i}", tag="vaug")
                for i in range(NT)
            ]
            oT = [
                oT_pool.tile([P, N], bf16, name=f"oT{i}", tag="oTp") for i in range(CT)
            ]

            def mm_tile(name, dtype=f32):
                return psum.tile([P, N], dtype, name=name, tag="mm", bufs=2)

            # ---------------- phase 0: transpose x into xT ----------------
            xins = []
            for si in range(NT):
                xin = xin_pool.tile([P, N], bf16, name=f"xin{si}", tag="xin")
                nc.sync.dma_start(out=xin[:, :], in_=x_d[si * P : (si + 1) * P, :])
                xins.append(xin)
            for si in range(NT):
                xin = xins[si]
                tr_ps = mm_tile(f"tr{si}", bf16)
                for ci in range(CT):
                    nc.tensor.transpose(
                        tr_ps[:, ci * P : (ci + 1) * P],
                        xin[:, ci * P : (ci + 1) * P],
                        identity,
                    )
                for ci in range(CT):
                    nc.vector.tensor_copy(
                        xT[ci][:, si * P : (si + 1) * P],
                        tr_ps[:, ci * P : (ci + 1) * P],
                    )

            # ---- interleavable q/k projection generator (one pair) ----
            def pair_proj_gen(t, qTt, kTt):
                """Yields once per 512-wide accumulation step (32 total);
                prefetches all weight chunks at creation; finishes with the
                PSUM->SBUF casts emitted between sub-phases."""
                wtiles = {}
                for which, colbase in (("q", t * P), ("k", C + t * P)):
                    # all 8 contraction chunks of this weight column block in
                    # one strided DMA (row ci*128+p, col c -> [p, ci, c])
                    w = wqk_pool.tile([P, CT * P], bf16, name=f"w{which}{t}", tag="wqk")
                    nc.sync.dma_start(
                        out=w.rearrange("p (ci c) -> p ci c", c=P),
                        in_=wqkv_d[:, colbase : colbase + P].rearrange(
                            "(ci p) c -> p ci c", p=P
                        ),
                    )
                    wtiles[which] = w
                for which, dst in (("q", qTt), ("k", kTt)):
                    for sh in range(HF):
                        sl = slice(sh * 512, sh * 512 + 512)
                        ps = psum.tile(
                            [P, 512], f32, name=f"pj{t}{which}{sh}", tag="proj", bufs=2
                        )
                        for ci in range(CT):
                            nc.tensor.matmul(
                                ps[:, :],
                                wtiles[which][:, ci * P : (ci + 1) * P],
                                xT[ci][:, sl],
                                start=(ci == 0),
                                stop=(ci == CT - 1),
                            )
                            yield
                        nc.vector.tensor_copy(dst[:, sl], ps[:, :])

            def new_pair(t):
                qTt = qkT_pool.tile([P, N], bf16, name=f"qT{t}", tag="qkT")
                kTt = qkT_pool.tile([P, N], bf16, name=f"kT{t}", tag="qkT")
                return qTt, kTt, pair_proj_gen(t, qTt, kTt)

            # -------- phase 1: v projection (+ pair-0 q/k interleaved) --------
            wv = []
            for ci in range(CT):
                w = wv_pool.tile([P, N], bf16, name=f"wv{ci}", tag="wv")
                nc.sync.dma_start(
                    out=w[:, :], in_=wqkv_d[ci * P : (ci + 1) * P, 2 * C : 3 * C]
                )
                wv.append(w)

            qT_cur, kT_cur, gen0 = new_pair(0)
            for sc in range(NT):
                v_ps = mm_tile(f"vps{sc}")
                for ci in range(CT):
                    st = dict(start=(ci == 0), stop=(ci == CT - 1))
                    for hf in range(HF):
                        sl = slice(hf * 512, hf * 512 + 512)
                        nc.tensor.matmul(
                            v_ps[:, sl],
                            xT[ci][:, sc * P : (sc + 1) * P],
                            wv[ci][:, sl],
                            **st,
                        )
                    next(gen0, None)
                va3 = vaug[sc].rearrange("p (h u) -> p h u", u=D + 1)
                nc.vector.tensor_copy(
                    va3[:, :, D : D + 1],
                    ones_f32[:, 0:H].rearrange("p (h u) -> p h u", u=1),
                )
                nc.vector.tensor_copy(
                    va3[:, :, 0:D],
                    v_ps.rearrange("p (h u) -> p h u", u=D),
                )
            for _ in gen0:  # drain any remaining projection steps
                pass

            # ---------------- phase 2: attention ----------------
            wos = []

            def prefetch_wo(ci):
                wo = wo_pool.tile([P, N], bf16, name=f"wo{ci}", tag="wo")
                nc.sync.dma_start(out=wo[:, :], in_=wout_d[ci * P : (ci + 1) * P, :])
                wos.append(wo)

            def normalize(h, t, row0, accs):
                # stage the accumulator out of PSUM (frees the acc banks),
                # then 1/rowsum (approx) -> partition-broadcast -> multiply.
                oTu = oTu_pool.tile([D + 1, N], f32, name=f"oTu{h}", tag="oTu")
                rs = rs_pool.tile([1, N], f32, name=f"rs{h}", tag="rsum")
                for hf in range(HF):
                    sl = slice(hf * 512, hf * 512 + 512)
                    nc.vector.tensor_copy(oTu[0 : D + 1, sl], accs[hf][0 : D + 1, :])
                    nc.vector.tensor_copy(rs[0:1, sl], accs[hf][D : D + 1, :])
                rc = rc_pool.tile([1, N], f32, name=f"rc{h}", tag="recip")
                nc.vector.reciprocal_approx_fast(rc[0:1, :], rs[0:1, :])
                bcs = bcs_pool.tile([D, N], f32, name=f"bcs{h}", tag="bcs")
                nc.gpsimd.partition_broadcast(bcs[0:D, :], rc[0:1, :], channels=D)
                nc.vector.tensor_mul(
                    oT[t][row0 : row0 + D, :], oTu[0:D, :], bcs[0:D, :]
                )

            av_queue = deque()  # (emit_fn, head_done_fn | None)

            def drain_av(n):
                for _ in range(n):
                    if not av_queue:
                        return
                    emit, done = av_queue.popleft()
                    emit()
                    if done is not None:
                        done()

            for t in range(HP):
                prefetch_wo(t)
                if t + 1 < HP:
                    qT_nxt, kT_nxt, gen = new_pair(t + 1)
                else:
                    qT_nxt = kT_nxt = gen = None
                for j in range(2):
                    h = 2 * t + j
                    row0 = D * j
                    accs = [
                        psum.tile([P, 512], f32, name=f"acc{h}_{hf}", tag="acc", bufs=2)
                        for hf in range(HF)
                    ]

                    def make_av(kc, pt, accs=accs, h=h):
                        def emit():
                            for hf in range(HF):
                                sl = slice(hf * 512, hf * 512 + 512)
                                nc.tensor.matmul(
                                    accs[hf][0 : D + 1, :],
                                    vaug[kc][:, h * (D + 1) : (h + 1) * (D + 1)],
                                    pt[:, sl],
                                    start=(kc == 0),
                                    stop=(kc == NT - 1),
                                )

                        return emit

                    done_fn = (
                        lambda h=h, t=t, row0=row0, accs=accs: normalize(
                            h, t, row0, accs
                        )
                    )
                    for kc in range(NT):
                        sT = mm_tile(f"s{h}_{kc}")
                        for hf in range(HF):
                            sl = slice(hf * 512, hf * 512 + 512)
                            nc.tensor.matmul(
                                sT[:, sl],
                                kT_cur[row0 : row0 + D, kc * P : (kc + 1) * P],
                                qT_cur[row0 : row0 + D, sl],
                                start=True,
                                stop=True,
                            )
                        pt = pT_pool.tile([P, N], bf16, name=f"pt{h}_{kc}", tag="pT")
                        nc.scalar.activation(
                            out=pt[:, :], in_=sT[:, :], func=Exp, scale=SCALE
                        )
                        av_queue.append(
                            (make_av(kc, pt), done_fn if kc == NT - 1 else None)
                        )
                        if len(av_queue) > LAG:
                            drain_av(len(av_queue) - LAG)
                        if gen is not None:
                            # 32 projection steps per pair over 16 slots
                            next(gen, None)
                            next(gen, None)
                if gen is not None:
                    for _ in gen:
                        pass
                qT_cur, kT_cur = qT_nxt, kT_nxt
            drain_av(len(av_queue))  # last head's AV tail + its normalize

            # ---------------- phase 3: out = o @ w_out + b ----------------
            for sc in range(NT):
                o_ps = mm_tile(f"ops{sc}")
                for ci in range(CT):
                    for hf in range(HF):
                        sl = slice(hf * 512, hf * 512 + 512)
                        nc.tensor.matmul(
                            o_ps[:, sl],
                            oT[ci][:, sc * P : (sc + 1) * P],
                            wos[ci][:, sl],
                            start=(ci == 0),
                            stop=(ci == CT - 1),
                        )
                ot = io_pool.tile([P, C], f32, name=f"ot{sc}", tag="ot")
                nc.vector.tensor_add(ot[:, :], o_ps[:, :], bias_bc[:, :])
                nc.sync.dma_start(out=out_d[sc * P : (sc + 1) * P, :], in_=ot[:, :])

    nc.compile()
    return nc


def _get_program():
    if "nc" not in _CACHE:
        _CACHE["nc"] = _build_program()
    return _CACHE["nc"]


def _bf16(a):
    import ml_dtypes

    return np.ascontiguousarray(np.asarray(a, dtype=np.float32)).astype(
        ml_dtypes.bfloat16
    )


def _in_maps(inputs):
    x = _bf16(inputs["x"])
    w_qkv = _bf16(inputs["w_qkv"])
    w_out = _bf16(inputs["w_out"])
    b_row = _bf16(np.asarray(inputs["b_out"]).reshape(1, C))
    return [
        {"x": x[i], "w_qkv": w_qkv, "w_out": w_out, "b_out": b_row} for i in range(B)
    ]


def kernel(x, w_qkv, w_out, b_out):
    from concourse.bass_utils import run_bass_kernel_spmd

    nc = _get_program()
    in_maps = _in_maps({"x": x, "w_qkv": w_qkv, "w_out": w_out, "b_out": b_out})
    res = run_bass_kernel_spmd(nc, in_maps, core_ids=list(range(B))).results
    return np.stack([res[i]["out"] for i in range(B)], axis=0)
